# revision 15
# baseline (speedup 1.0000x reference)
"""v8d: 2 merged-pair pipelines (4 time-subsegments as 2 units of 2),
with the tanh/h-mul stage skewed one iteration behind the gate stage.

Unit u holds subsegments (2u, 2u+1); a slot is 512 cols = [p0 256 | p1 256]
(each half is the full 256-batch for one subsegment).

Per unit u, per iteration k:
  ACT : tau = tanh(c(k-1) [64,512])          (skewed: c from previous iter)
  DVE : h(k-1) = s_o(k-1) * tau -> mega slot k rows 0:64
  mm1 : z[:, 0:512]    = [Wf | Wi]^T @ xh_slot(k)   (f top, i bottom)
  mm2 : z[:, 512:1024] = [Wo | 2Wg]^T @ xh_slot(k)  (o top, g2 bottom)
  ACT : s = sigmoid(z [128,1024]) -> f16     (tanh(x) = 2 sig(2x) - 1)
  DVE : m  = (s_g2 - 0.5) * s_i -> cm[64:128, 0:512]
  DVE : cf = s_f * c(k-1)       -> cm[0:64, 0:512]
  mm3 : c(k) = [I; 2I]^T @ cm -> c-psum [64,512] f32  (= f*c + i*g)

The skew lets every ACT op start with its input already computed: the
ACT queue per iteration is [tanh, tanh, sig, sig] with no bubbles, and
the cell-state ladder (stt/cf/mm3) has a full iteration of slack before
its tanh consumes it. c is double-buffered in PSUM (2 x [64,1024] f32).

Mega tiles hold 8 slots ([97, 8*512] f16): rows 64:97 = [y;1] (one DMA per
8 steps), rows 0:64 = h (slot u holds h_{u-1}). Out streams mega[0:64] once
per 8 slots as [H, 8*512] f16; host transposes/casts."""

import numpy as np

import concourse.bacc as bacc
import concourse.mybir as mybir
from concourse.bass_utils import run_bass_kernel_spmd
from concourse.alu_op_type import AluOpType
from concourse.tile import TileContext

F32 = mybir.dt.float32
F16 = mybir.dt.float16

B_TOTAL = 256
T_FULL = 2048
D = 32
H = 64
N_CORES = 8
HB = 128
N_SUB = 8            # time-subsegments per core
NM = 4               # merged units (2 subsegments each)
PB = 2 * HB          # 256 batch cols per subsegment slot-half
UW = 2 * PB          # 512 cols per merged-unit slot
WARM = 16
K_ST = H + D + 1
SLOTS = 8

SIG = mybir.ActivationFunctionType.Sigmoid
TANH = mybir.ActivationFunctionType.Tanh


def _derive(seg_sub, warm):
    S = seg_sub + warm
    n_meg = (S + 1 + SLOTS - 1) // SLOTS
    SW = n_meg * SLOTS
    return S, n_meg, SW


def build_nc(seg_sub, warm=WARM):
    S, n_meg, SW = _derive(seg_sub, warm)

    nc = bacc.Bacc()
    yT = nc.dram_tensor("yT", [D + 1, NM * SW * UW], F16, kind="ExternalInput")
    wp = nc.dram_tensor("wp", [K_ST, 2 * HB], F16, kind="ExternalInput")
    ii2d = nc.dram_tensor("ii2", [2 * H, H], F16, kind="ExternalInput")
    out = nc.dram_tensor("out", [H, NM * SW * UW], F16, kind="ExternalOutput")

    with TileContext(nc) as tc:
        with (
            tc.tile_pool(name="const", bufs=1) as cons,
            tc.tile_pool(name="mega", bufs=2) as mp,
            tc.tile_pool(name="spool", bufs=2) as sp,
            tc.tile_pool(name="cmpool", bufs=2) as cmp_,
            tc.tile_pool(name="taupool", bufs=2) as tp,
            tc.tile_pool(name="zpsum", bufs=2, space="PSUM") as zp,
            tc.tile_pool(name="cpsum", bufs=1, space="PSUM") as cp,
        ):
            wpt = cons.tile([K_ST, 2 * HB], F16)
            nc.sync.dma_start(wpt, wp[:, :])
            ii2 = cons.tile([2 * H, H], F16)
            nc.sync.dma_start(ii2, ii2d[:, :])

            def new_mega(u, i):
                t = mp.tile([K_ST, SLOTS * UW], F16, tag=f"meg{u}",
                            name=f"meg{u}_{i}")
                base = (u * SW + i * SLOTS) * UW
                nc.sync.dma_start(t[H:K_ST, :], yT[:, base : base + SLOTS * UW])
                return t

            megas = [[new_mega(u, 0), new_mega(u, 1)] for u in range(NM)]
            cprev, sprev = [], [None] * NM
            for u in range(NM):
                nc.vector.memset(megas[u][0][0:H, 0:UW], 0.0)
                c0 = cp.tile([H, UW], F32, tag=f"c{u}", name=f"c{u}_init")
                nc.vector.memset(c0, 0.0)
                cprev.append(c0)

            for k in range(S + 1):
                sl = k % SLOTS
                # --- skewed stage: tanh + h for step k-1 into slot k ---
                if k > 0:
                    taus = []
                    for u in range(NM):
                        tau = tp.tile([H, UW], F16, tag=f"tau{u}",
                                      name=f"tau{u}_{k}")
                        nc.scalar.activation(tau, cprev[u], TANH)
                        taus.append(tau)
                    for u in range(NM):
                        nc.vector.tensor_mul(
                            megas[u][0][0:H, sl * UW : (sl + 1) * UW],
                            sprev[u][0:H, UW : 2 * UW],
                            taus[u],
                        )
                if k == S:
                    break
                # --- gate stage for step k ---
                zs = []
                for u in range(NM):
                    xh = megas[u][0][:, sl * UW : (sl + 1) * UW]
                    z = zp.tile([2 * H, 2 * UW], F32, tag="z", name=f"z{u}_{k}")
                    nc.tensor.matmul(z[:, 0:UW], wpt[:, 0:HB], xh,
                                     start=True, stop=True, skip_group_check=True)
                    nc.tensor.matmul(z[:, UW : 2 * UW], wpt[:, HB : 2 * HB], xh,
                                     start=True, stop=True, skip_group_check=True)
                    zs.append(z)
                ss = []
                for u in range(NM):
                    s = sp.tile([2 * H, 2 * UW], F16, tag=f"s{u}", name=f"s{u}_{k}")
                    nc.scalar.activation(s, zs[u], SIG)
                    ss.append(s)
                for u in range(NM):
                    s = ss[u]
                    cm = cmp_.tile([2 * H, UW], F16, tag=f"cm{u}", name=f"cm{u}_{k}")
                    nc.vector.scalar_tensor_tensor(
                        cm[H : 2 * H, :], s[H : 2 * H, UW : 2 * UW], 0.5,
                        s[H : 2 * H, 0:UW],
                        AluOpType.subtract, AluOpType.mult,
                    )
                    nc.vector.tensor_mul(cm[0:H, :], s[0:H, 0:UW], cprev[u])
                    cnew = cp.tile([H, UW], F32, tag=f"c{u}", name=f"c{u}_{k}")
                    nc.tensor.matmul(cnew, ii2, cm,
                                     start=True, stop=True, skip_group_check=True)
                    cprev[u] = cnew
                    sprev[u] = s

                if sl == SLOTS - 1:
                    mi = k // SLOTS
                    for u in range(NM):
                        full = megas[u][0]
                        base = (u * SW + mi * SLOTS) * UW
                        nc.sync.dma_start(
                            out[:, base : base + SLOTS * UW], full[0:H, :]
                        )
                        megas[u][0] = megas[u][1]
                        nmi = mi + 2
                        megas[u][1] = (
                            new_mega(u, nmi) if nmi < n_meg else megas[u][0]
                        )

            mi_last = S // SLOTS
            used = S % SLOTS + 1
            for u in range(NM):
                base = (u * SW + mi_last * SLOTS) * UW
                nc.sync.dma_start(
                    out[:, base : base + used * UW],
                    megas[u][0][0:H, 0 : used * UW],
                )

    nc.finalize()
    return nc


def _prep_inputs(y, Wx, Wh, b, seg_sub, warm=WARM):
    S, n_meg, SW = _derive(seg_sub, warm)
    y = np.asarray(y, dtype=np.float32)
    T = y.shape[1]
    seg_core = T // N_CORES

    wcat = np.concatenate(
        [np.asarray(Wh), np.asarray(Wx), np.asarray(b).reshape(1, 4 * H)], axis=0
    ).astype(np.float32)
    gi = wcat[:, 0:H]
    gf = wcat[:, H : 2 * H]
    gg = wcat[:, 2 * H : 3 * H]
    go = wcat[:, 3 * H : 4 * H]
    wpair = np.concatenate([gf, gi, go, 2.0 * gg], axis=1).astype(np.float16)

    ii2 = np.concatenate(
        [np.eye(H, dtype=np.float16), 2.0 * np.eye(H, dtype=np.float16)], axis=0
    )

    yx = np.concatenate(
        [y.transpose(2, 1, 0).astype(np.float16),
         np.ones((1, T, B_TOTAL), np.float16)], axis=0)  # [33, T, 256]
    in_maps = []
    for c in range(N_CORES):
        yTc = np.zeros((D + 1, NM, SW, 2, PB), np.float16)
        for q in range(N_SUB):
            u, hhalf = q // 2, q % 2
            t0 = c * seg_core + q * seg_sub - warm
            lo = max(t0, 0)
            hi = min(t0 + S, T)
            if hi > lo:
                yTc[:, u, lo - t0 : hi - t0, hhalf, :] = (
                    yx[:, lo:hi, :]
                )
        in_maps.append({
            "yT": np.ascontiguousarray(yTc.reshape(D + 1, NM * SW * UW)),
            "wp": wpair,
            "ii2": ii2,
        })
    return in_maps


def _unshard(results, seg_sub, warm=WARM):
    S, n_meg, SW = _derive(seg_sub, warm)
    T = T_FULL
    seg_core = T // N_CORES
    full = np.empty((B_TOTAL, T, H), np.float32)
    for c in range(N_CORES):
        o = results[c]["out"].reshape(H, NM, SW, 2, PB)
        for q in range(N_SUB):
            u, hhalf = q // 2, q % 2
            blk = o[:, u, warm + 1 : S + 1, hhalf, :].astype(np.float32)
            t0 = c * seg_core + q * seg_sub
            n = min(t0 + seg_sub, T) - t0
            full[:, t0 : t0 + n, :] = blk.transpose(2, 1, 0)[:, :n, :]
    return full


_NC_CACHE = {}


def kernel(y, Wx, Wh, b):
    T = y.shape[1]
    seg_sub = -(-(T // N_CORES) // N_SUB)
    key = (seg_sub, WARM)
    if key not in _NC_CACHE:
        _NC_CACHE[key] = build_nc(seg_sub, WARM)
    nc = _NC_CACHE[key]
    in_maps = _prep_inputs(y, Wx, Wh, b, seg_sub, WARM)
    res = run_bass_kernel_spmd(nc, in_maps, core_ids=list(range(N_CORES)))
    return _unshard(res.results, seg_sub, WARM)


# revision 16
# speedup vs baseline: 1.1271x; 1.1271x over previous
"""v9: 3 merged-pair pipelines (6 time-subsegments as 3 units of 2),
with the tanh/h-mul stage skewed one iteration behind the gate stage.

Unit u holds subsegments (2u, 2u+1); a slot is 512 cols = [p0 256 | p1 256]
(each half is the full 256-batch for one subsegment).

Per unit u, per iteration k:
  ACT : tau = tanh(c(k-1) [64,512])          (skewed: c from previous iter)
  DVE : h(k-1) = s_o(k-1) * tau -> mega slot k rows 0:64
  mm1 : z[:, 0:512]    = [Wf | Wi]^T @ xh_slot(k)   (f top, i bottom)
  mm2 : z[:, 512:1024] = [Wo | 2Wg]^T @ xh_slot(k)  (o top, g2 bottom)
  ACT : s = sigmoid(z [128,1024]) -> f16     (tanh(x) = 2 sig(2x) - 1)
  DVE : m  = (s_g2 - 0.5) * s_i -> cm[64:128, 0:512]
  DVE : cf = s_f * c(k-1)       -> cm[0:64, 0:512]
  mm3 : c(k) = [I; 2I]^T @ cm -> c-psum [64,512] f32  (= f*c + i*g)

The skew lets every ACT op start with its input already computed: the
ACT queue per iteration is [tanh, tanh, sig, sig] with no bubbles, and
the cell-state ladder (stt/cf/mm3) has a full iteration of slack before
its tanh consumes it. c is single-buffered in PSUM ([64,512] f32 per
unit; WAR edges via the tanh/cf reads order the in-place mm3 update),
and the three units rotate through a shared 2-buffer z pool (4 banks).

Mega tiles hold 8 slots ([97, 8*512] f16): rows 64:97 = [y;1] (one DMA per
8 steps), rows 0:64 = h (slot u holds h_{u-1}). Out streams mega[0:64] once
per 8 slots as [H, 8*512] f16; host transposes/casts."""

import numpy as np

import concourse.bacc as bacc
import concourse.mybir as mybir
from concourse.bass_utils import run_bass_kernel_spmd
from concourse.alu_op_type import AluOpType
from concourse.tile import TileContext

F32 = mybir.dt.float32
F16 = mybir.dt.float16

B_TOTAL = 256
T_FULL = 2048
D = 32
H = 64
N_CORES = 8
HB = 128
N_SUB = 6            # time-subsegments per core
NM = 3               # merged units (2 subsegments each)
PB = 2 * HB          # 256 batch cols per subsegment slot-half
UW = 2 * PB          # 512 cols per merged-unit slot
WARM = 14
K_ST = H + D + 1
SLOTS = 8

SIG = mybir.ActivationFunctionType.Sigmoid
TANH = mybir.ActivationFunctionType.Tanh


def _derive(seg_sub, warm):
    S = seg_sub + warm
    n_meg = (S + 1 + SLOTS - 1) // SLOTS
    SW = n_meg * SLOTS
    return S, n_meg, SW


def build_nc(seg_sub, warm=WARM):
    S, n_meg, SW = _derive(seg_sub, warm)

    nc = bacc.Bacc()
    yT = nc.dram_tensor("yT", [D + 1, NM * SW * UW], F16, kind="ExternalInput")
    wp = nc.dram_tensor("wp", [K_ST, 2 * HB], F16, kind="ExternalInput")
    ii2d = nc.dram_tensor("ii2", [2 * H, H], F16, kind="ExternalInput")
    out = nc.dram_tensor("out", [H, NM * SW * UW], F16, kind="ExternalOutput")

    with TileContext(nc) as tc:
        with (
            tc.tile_pool(name="const", bufs=1) as cons,
            tc.tile_pool(name="mega", bufs=2) as mp,
            tc.tile_pool(name="spool", bufs=2) as sp,
            tc.tile_pool(name="cmpool", bufs=2) as cmp_,
            tc.tile_pool(name="taupool", bufs=2) as tp,
            tc.tile_pool(name="zpsum", bufs=2, space="PSUM") as zp,
            tc.tile_pool(name="cpsum", bufs=1, space="PSUM") as cp,
        ):
            wpt = cons.tile([K_ST, 2 * HB], F16)
            nc.sync.dma_start(wpt, wp[:, :])
            ii2 = cons.tile([2 * H, H], F16)
            nc.sync.dma_start(ii2, ii2d[:, :])

            def new_mega(u, i):
                t = mp.tile([K_ST, SLOTS * UW], F16, tag=f"meg{u}",
                            name=f"meg{u}_{i}")
                base = (u * SW + i * SLOTS) * UW
                nc.sync.dma_start(t[H:K_ST, :], yT[:, base : base + SLOTS * UW])
                return t

            megas = [[new_mega(u, 0), new_mega(u, 1)] for u in range(NM)]
            cprev, sprev = [], [None] * NM
            for u in range(NM):
                nc.vector.memset(megas[u][0][0:H, 0:UW], 0.0)
                c0 = cp.tile([H, UW], F32, tag=f"c{u}", name=f"c{u}_init")
                nc.vector.memset(c0, 0.0)
                cprev.append(c0)

            for k in range(S + 1):
                sl = k % SLOTS
                # --- skewed stage: tanh + h for step k-1 into slot k ---
                if k > 0:
                    taus = []
                    for u in range(NM):
                        tau = tp.tile([H, UW], F16, tag=f"tau{u}",
                                      name=f"tau{u}_{k}")
                        nc.scalar.activation(tau, cprev[u], TANH)
                        taus.append(tau)
                    for u in range(NM):
                        nc.vector.tensor_mul(
                            megas[u][0][0:H, sl * UW : (sl + 1) * UW],
                            sprev[u][0:H, UW : 2 * UW],
                            taus[u],
                        )
                if k == S:
                    break
                # --- gate stage for step k ---
                zs = []
                for u in range(NM):
                    xh = megas[u][0][:, sl * UW : (sl + 1) * UW]
                    z = zp.tile([2 * H, 2 * UW], F32, tag="z", name=f"z{u}_{k}")
                    nc.tensor.matmul(z[:, 0:UW], wpt[:, 0:HB], xh,
                                     start=True, stop=True, skip_group_check=True)
                    nc.tensor.matmul(z[:, UW : 2 * UW], wpt[:, HB : 2 * HB], xh,
                                     start=True, stop=True, skip_group_check=True)
                    zs.append(z)
                ss = []
                for u in range(NM):
                    s = sp.tile([2 * H, 2 * UW], F16, tag=f"s{u}", name=f"s{u}_{k}")
                    nc.scalar.activation(s, zs[u], SIG)
                    ss.append(s)
                for u in range(NM):
                    s = ss[u]
                    cm = cmp_.tile([2 * H, UW], F16, tag=f"cm{u}", name=f"cm{u}_{k}")
                    nc.vector.scalar_tensor_tensor(
                        cm[H : 2 * H, :], s[H : 2 * H, UW : 2 * UW], 0.5,
                        s[H : 2 * H, 0:UW],
                        AluOpType.subtract, AluOpType.mult,
                    )
                    nc.vector.tensor_mul(cm[0:H, :], s[0:H, 0:UW], cprev[u])
                    cnew = cp.tile([H, UW], F32, tag=f"c{u}", name=f"c{u}_{k}")
                    nc.tensor.matmul(cnew, ii2, cm,
                                     start=True, stop=True, skip_group_check=True)
                    cprev[u] = cnew
                    sprev[u] = s

                if sl == SLOTS - 1:
                    mi = k // SLOTS
                    for u in range(NM):
                        full = megas[u][0]
                        base = (u * SW + mi * SLOTS) * UW
                        nc.sync.dma_start(
                            out[:, base : base + SLOTS * UW], full[0:H, :]
                        )
                        megas[u][0] = megas[u][1]
                        nmi = mi + 2
                        megas[u][1] = (
                            new_mega(u, nmi) if nmi < n_meg else megas[u][0]
                        )

            mi_last = S // SLOTS
            used = S % SLOTS + 1
            for u in range(NM):
                base = (u * SW + mi_last * SLOTS) * UW
                nc.sync.dma_start(
                    out[:, base : base + used * UW],
                    megas[u][0][0:H, 0 : used * UW],
                )

    nc.finalize()
    return nc


def _prep_inputs(y, Wx, Wh, b, seg_sub, warm=WARM):
    S, n_meg, SW = _derive(seg_sub, warm)
    y = np.asarray(y, dtype=np.float32)
    T = y.shape[1]
    seg_core = T // N_CORES

    wcat = np.concatenate(
        [np.asarray(Wh), np.asarray(Wx), np.asarray(b).reshape(1, 4 * H)], axis=0
    ).astype(np.float32)
    gi = wcat[:, 0:H]
    gf = wcat[:, H : 2 * H]
    gg = wcat[:, 2 * H : 3 * H]
    go = wcat[:, 3 * H : 4 * H]
    wpair = np.concatenate([gf, gi, go, 2.0 * gg], axis=1).astype(np.float16)

    ii2 = np.concatenate(
        [np.eye(H, dtype=np.float16), 2.0 * np.eye(H, dtype=np.float16)], axis=0
    )

    yx = np.concatenate(
        [y.transpose(2, 1, 0).astype(np.float16),
         np.ones((1, T, B_TOTAL), np.float16)], axis=0)  # [33, T, 256]
    in_maps = []
    for c in range(N_CORES):
        yTc = np.zeros((D + 1, NM, SW, 2, PB), np.float16)
        for q in range(N_SUB):
            u, hhalf = q // 2, q % 2
            t0 = c * seg_core + q * seg_sub - warm
            lo = max(t0, 0)
            hi = min(t0 + S, T)
            if hi > lo:
                yTc[:, u, lo - t0 : hi - t0, hhalf, :] = (
                    yx[:, lo:hi, :]
                )
        in_maps.append({
            "yT": np.ascontiguousarray(yTc.reshape(D + 1, NM * SW * UW)),
            "wp": wpair,
            "ii2": ii2,
        })
    return in_maps


def _unshard(results, seg_sub, warm=WARM):
    S, n_meg, SW = _derive(seg_sub, warm)
    T = T_FULL
    seg_core = T // N_CORES
    full = np.empty((B_TOTAL, T, H), np.float32)
    for c in range(N_CORES):
        o = results[c]["out"].reshape(H, NM, SW, 2, PB)
        for q in range(N_SUB):
            u, hhalf = q // 2, q % 2
            blk = o[:, u, warm + 1 : S + 1, hhalf, :].astype(np.float32)
            t0 = c * seg_core + q * seg_sub
            n = min(t0 + seg_sub, T) - t0
            full[:, t0 : t0 + n, :] = blk.transpose(2, 1, 0)[:, :n, :]
    return full


_NC_CACHE = {}


def kernel(y, Wx, Wh, b):
    T = y.shape[1]
    seg_sub = -(-(T // N_CORES) // N_SUB)
    key = (seg_sub, WARM)
    if key not in _NC_CACHE:
        _NC_CACHE[key] = build_nc(seg_sub, WARM)
    nc = _NC_CACHE[key]
    in_maps = _prep_inputs(y, Wx, Wh, b, seg_sub, WARM)
    res = run_bass_kernel_spmd(nc, in_maps, core_ids=list(range(N_CORES)))
    return _unshard(res.results, seg_sub, WARM)
